# revision 3
# baseline (speedup 1.0000x reference)
"""Trainium2 Bass kernel for BroadcastResidualBlock.

Reference computation (per image, NHWC, H=W=19, C=256, HW=361):
    h1 = relu(bn1(x @ conv1_w + conv1_b))          # 1x1 conv = channel mix
    h2 = relu(dense(h1 over flattened board))       # spatial mix, per channel
    h3 = relu(bn2(h2 @ conv2_w + conv2_b))          # 1x1 conv
    out = x + h3

Strategy: pure data parallel over batch N=256 -> 32 images per core on 8
cores.  BN (inference) folds into the conv weights/biases on the host.  The
host also pre-transposes x into "C-layout" (N, C, HW) so every device-side
matmul contracts over the partition dimension with zero on-device transposes:

    s1: psum[r,  d] += xC_bf16[c_chunk, r_chunk].T @ w1[c_chunk, d]   (h1: S-layout)
    s2: psum[c,  q] += h1[p_chunk, c_chunk].T     @ dw[p_chunk, q]    (h2: C-layout)
    s3: psum[d,  q] += w2[c_chunk, d_chunk].T     @ h2[c_chunk, q]    (h3: C-layout)
    out = relu(psum3) + xC   (single fused VectorE op), stored in C-layout.

Matmuls run in bf16 (fp32 PSUM accumulation); x stays fp32 for the residual.
The host transposes the output back to NHWC.

Per-core steady state (cost-model): PE ~72us, DMA ~67us, ACT ~49us, DVE ~41us.
DMAs are batched 4 images per transfer and all weights ship as one blob so the
HWDGE/sequencer fixed costs (~625ns+900ns per DMA) stay off the critical path.
Each stage's PSUM lives in one 2-bank [128, 1024] tile so the whole epilogue
of a stage is a single DVE/ACT instruction.
"""

import numpy as np
import ml_dtypes

import concourse.bass as bass
import concourse.mybir as mybir
import concourse.tile as tile
from concourse import bacc
from concourse.bass_utils import run_bass_kernel_spmd

N_CORES = 8
NIMG = 32            # images per core
B = 4                # images per DMA batch
C = 256
HW = 361             # 19*19
P = 128
EPS = 1e-3
W_COLS = 2 * C + 3 * HW + 2 * C  # weight blob free size: w1 | dw | w2

F32 = mybir.dt.float32
BF16 = mybir.dt.bfloat16
AF = mybir.ActivationFunctionType
ALU = mybir.AluOpType

_prog_cache = {}


def build_program(has_b1: bool, has_b2: bool, has_b3: bool, reps: int = 1):
    nc = bacc.Bacc("TRN2", target_bir_lowering=False, debug=False)

    xc = nc.dram_tensor("xc", [NIMG, 2, P, HW], F32, kind="ExternalInput").ap()
    wb = nc.dram_tensor("wb", [P, W_COLS], BF16, kind="ExternalInput").ap()
    b1 = b2 = b3 = None
    if has_b1:
        b1 = nc.dram_tensor("b1", [P, 3 * C], F32, kind="ExternalInput").ap()
    if has_b2:
        b2 = nc.dram_tensor("b2", [P, 2, HW], F32, kind="ExternalInput").ap()
    if has_b3:
        b3 = nc.dram_tensor("b3", [2, P], F32, kind="ExternalInput").ap()
    yc = nc.dram_tensor("yc", [NIMG, 2, P, HW], F32, kind="ExternalOutput").ap()

    with tile.TileContext(nc) as tc:
        with (
            tc.tile_pool(name="const", bufs=1) as cpool,
            tc.tile_pool(name="xf", bufs=3) as xf_pool,
            tc.tile_pool(name="xb", bufs=3) as xb_pool,
            tc.tile_pool(name="h1", bufs=3) as h1_pool,
            tc.tile_pool(name="h2", bufs=3) as h2_pool,
            tc.tile_pool(name="yo", bufs=2) as yo_pool,
            tc.tile_pool(name="ps", bufs=4, space="PSUM") as ps_pool,
        ):
            wsb = cpool.tile([P, W_COLS], BF16)
            nc.sync.dma_start(wsb[:], wb)
            # views into the weight blob
            O_DW = 2 * C
            O_W2 = 2 * C + 3 * HW

            def w1_ap(cc):                      # [128, 256] rhs for s1
                return wsb[:, cc * C : (cc + 1) * C]

            def dw_ap(pc, k):                   # [k, 361] rhs for s2
                return wsb[:k, O_DW + pc * HW : O_DW + (pc + 1) * HW]

            def w2_ap(cc, dc):                  # [128, 128] lhsT for s3
                o = O_W2 + cc * C + dc * P
                return wsb[:, o : o + P]

            b1sb = b2sb = b3sb = None
            if has_b1:
                b1sb = cpool.tile([P, 3 * C], F32)
                nc.sync.dma_start(b1sb[:], b1)
            if has_b2:
                b2sb = cpool.tile([P, 2, HW], F32)
                nc.sync.dma_start(b2sb[:], b2)
            if has_b3:
                b3sb = cpool.tile([P, 2], F32)
                nc.sync.dma_start(b3sb[:], b3.rearrange("co ci -> ci co"))

            def emit_load(bi):
                xf = xf_pool.tile([P, B, 2, HW], F32, tag="xf")
                nc.sync.dma_start(
                    xf[:], xc[bi * B : (bi + 1) * B].rearrange("n co ci q -> ci n co q"))
                xb = xb_pool.tile([P, B, 2, HW], BF16, tag="xb")
                nc.vector.tensor_copy(xb[:], xf[:])
                return xf, xb

            def emit_s1(i, xb):
                k = i % B
                h1 = h1_pool.tile([P, 3, C], BF16, tag="h1")
                ps = ps_pool.tile([P, 1024], F32, tag="ps")
                for rc in range(3):
                    m = 128 if rc < 2 else 105
                    for cc in range(2):
                        nc.tensor.matmul(
                            ps[:m, rc * C : rc * C + C],
                            xb[:, k, cc, rc * 128 : rc * 128 + m],
                            w1_ap(cc),
                            start=(cc == 0),
                            stop=(cc == 1),
                        )
                if b1sb is not None:
                    nc.vector.scalar_tensor_tensor(
                        ps[:, : 3 * C], ps[:, : 3 * C], 0.0, b1sb[:],
                        ALU.bypass, ALU.add)
                nc.scalar.activation(
                    h1[:].rearrange("p a b -> p (a b)"), ps[:, : 3 * C], AF.Relu)
                return h1

            def emit_s2(i, h1):
                h2 = h2_pool.tile([P, 2, HW], BF16, tag="h2")
                ps = ps_pool.tile([P, 1024], F32, tag="ps")
                for cc in range(2):
                    for pc in range(3):
                        k = 128 if pc < 2 else 105
                        nc.tensor.matmul(
                            ps[:, cc * 512 : cc * 512 + HW],
                            h1[:k, pc, cc * 128 : (cc + 1) * 128],
                            dw_ap(pc, k),
                            start=(pc == 0),
                            stop=(pc == 2),
                        )
                psv = ps.rearrange("p (c x) -> p c x", c=2)[:, :, :HW]
                if b2sb is not None:
                    nc.vector.scalar_tensor_tensor(
                        psv, psv, 0.0, b2sb[:], ALU.bypass, ALU.add)
                nc.scalar.activation(h2[:], psv, AF.Relu)
                return h2

            def emit_s3(i, xf, yo, h2):
                k = i % B
                ps = ps_pool.tile([P, 1024], F32, tag="ps")
                for dc in range(2):
                    for cc in range(2):
                        nc.tensor.matmul(
                            ps[:, dc * 512 : dc * 512 + HW],
                            w2_ap(cc, dc),
                            h2[:, cc, :],
                            start=(cc == 0),
                            stop=(cc == 1),
                        )
                psv = ps.rearrange("p (c x) -> p c x", c=2)[:, :, :HW]
                if b3sb is not None:
                    for dc in range(2):
                        nc.scalar.activation(
                            yo[:, k, dc, :], psv[:, dc, :], AF.Relu,
                            bias=b3sb[:, dc : dc + 1])
                    nc.vector.tensor_add(
                        yo[:, k, :, :], yo[:, k, :, :], xf[:, k, :, :])
                else:
                    nc.vector.scalar_tensor_tensor(
                        yo[:, k, :, :], psv, 0.0, xf[:, k, :, :],
                        ALU.max, ALU.add)

            def emit_store(bi, yo):
                nc.sync.dma_start(
                    yc[bi * B : (bi + 1) * B].rearrange("n co ci q -> ci n co q"),
                    yo[:])

            def body():
                # software pipeline: s1(i) | s2(i-1) | s3(i-2); batch loads
                # prefetched two steps ahead, stores flushed per batch
                xfs, xbs, h1s, h2s, yos = {}, {}, {}, {}, {}
                xfs[0], xbs[0] = emit_load(0)
                for step in range(NIMG + 2):
                    nb = (step + 2) // B
                    if (step + 2) % B == 0 and nb < NIMG // B:
                        xfs[nb], xbs[nb] = emit_load(nb)
                    if step >= 2:
                        i = step - 2
                        bi = i // B
                        if i % B == 0:
                            yos[bi] = yo_pool.tile(
                                [P, B, 2, HW], F32, tag="yo", name="yo")
                        emit_s3(i, xfs[bi], yos[bi], h2s.pop(i))
                        if i % B == B - 1:
                            emit_store(bi, yos.pop(bi))
                            xfs.pop(bi)
                    if 1 <= step <= NIMG:
                        h2s[step - 1] = emit_s2(step - 1, h1s.pop(step - 1))
                    if step < NIMG:
                        i = step
                        h1s[i] = emit_s1(i, xbs[i // B])
                        if i % B == B - 1:
                            xbs.pop(i // B)

            if reps == 1:
                body()
            else:
                with tc.For_i(0, reps, 1):
                    body()

    nc.compile()
    return nc


def _get_program(key):
    if key not in _prog_cache:
        _prog_cache[key] = build_program(*key)
    return _prog_cache[key]


def _marshal(x, conv1_w, conv1_b, bn1_mean, bn1_var, bn1_beta,
             dense_w, dense_b, conv2_w, conv2_b, bn2_mean, bn2_var, bn2_beta):
    bf16 = ml_dtypes.bfloat16
    n = x.shape[0]
    rs1 = 1.0 / np.sqrt(bn1_var.astype(np.float64) + EPS)
    rs2 = 1.0 / np.sqrt(bn2_var.astype(np.float64) + EPS)
    w1f = conv1_w.astype(np.float64) * rs1[None, :]
    w2f = conv2_w.astype(np.float64) * rs2[None, :]
    b1f = (conv1_b - bn1_mean).astype(np.float64) * rs1 + bn1_beta
    b2f = dense_b.astype(np.float64)
    b3f = (conv2_b - bn2_mean).astype(np.float64) * rs2 + bn2_beta
    has_b1 = bool(np.any(b1f != 0.0))
    has_b2 = bool(np.any(b2f != 0.0))
    has_b3 = bool(np.any(b3f != 0.0))

    # weight blob [128, W_COLS]: per partition ci the columns are
    #   w1[cc=0..1] (256 each) | dw[pc=0..2] (361 each) | w2[cc=0..1] (256 each)
    blob = np.zeros((P, W_COLS), np.float64)
    w1r = w1f.reshape(2, P, C)
    for cc in range(2):
        blob[:, cc * C : (cc + 1) * C] = w1r[cc]
    dwp = np.zeros((3 * P, HW), np.float64)
    dwp[:HW] = dense_w
    dwr = dwp.reshape(3, P, HW)
    for pc in range(3):
        blob[:, 2 * C + pc * HW : 2 * C + (pc + 1) * HW] = dwr[pc]
    w2r = w2f.reshape(2, P, C)
    for cc in range(2):
        blob[:, 2 * C + 3 * HW + cc * C : 2 * C + 3 * HW + (cc + 1) * C] = w2r[cc]
    wbb = blob.astype(bf16)

    x_c = np.ascontiguousarray(
        x.reshape(n, HW, C).transpose(0, 2, 1)
    ).reshape(N_CORES, NIMG, 2, P, HW)

    in_maps = []
    for c in range(N_CORES):
        m = {"xc": x_c[c], "wb": wbb}
        if has_b1:
            m["b1"] = np.ascontiguousarray(np.broadcast_to(
                np.tile(b1f, 3).astype(np.float32), (P, 3 * C)))
        if has_b2:
            m["b2"] = np.ascontiguousarray(np.broadcast_to(
                b2f.astype(np.float32), (P, 2, HW)))
        if has_b3:
            m["b3"] = np.ascontiguousarray(
                b3f.astype(np.float32).reshape(2, P))
        in_maps.append(m)
    return (has_b1, has_b2, has_b3), in_maps


def _unmarshal(results, n, h, w):
    y = np.stack([results[c]["yc"] for c in range(N_CORES)])
    y = y.reshape(n, C, HW).transpose(0, 2, 1)
    return np.ascontiguousarray(y.reshape(n, h, w, C).astype(np.float32))


def kernel(x, conv1_w, conv1_b, bn1_mean, bn1_var, bn1_beta,
           dense_w, dense_b, conv2_w, conv2_b, bn2_mean, bn2_var, bn2_beta):
    n, h, w, _ = x.shape
    flags, in_maps = _marshal(
        x, conv1_w, conv1_b, bn1_mean, bn1_var, bn1_beta,
        dense_w, dense_b, conv2_w, conv2_b, bn2_mean, bn2_var, bn2_beta)
    nc = _get_program((*flags, 1))
    res = run_bass_kernel_spmd(nc, in_maps, list(range(N_CORES)))
    return _unmarshal(res.results, n, h, w)
